# revision 39
# baseline (speedup 1.0000x reference)
import sys

sys.path.insert(0, "/opt/trn_rl_repo")

import numpy as np
from contextlib import ExitStack

import concourse.bass as bass
import concourse.bacc as bacc
import concourse.tile as tile
from concourse import mybir
from concourse.bass_utils import run_bass_kernel_spmd
from concourse.masks import make_identity

B, C, H, W = 16, 64, 64, 64
HW = H * W          # 4096
M = HW // 4         # 1024
NCORES = 8
BPC = B // NCORES   # batches per core
F32 = mybir.dt.float32
BF16 = mybir.dt.bfloat16
FP8 = mybir.dt.float8e4

NCHUNK = 1024       # n-dim chunk (columns of s^T / o)
NCH = HW // NCHUNK  # 4 chunks per batch
MT = M // 128       # 8 m-tiles of 128


def _build_nc():
    nc = bacc.Bacc(None, target_bir_lowering=False)

    x_d = nc.dram_tensor("x", [BPC, C, HW], F32, kind="ExternalInput")
    xb_d = nc.dram_tensor("xb", [BPC, C, HW], BF16, kind="ExternalInput")
    wpgt_d = nc.dram_tensor("wpgt", [C, 40], BF16, kind="ExternalInput")     # [w_g; w_phi]^T
    wtheta_d = nc.dram_tensor("wtheta", [8, C], BF16, kind="ExternalInput")  # lhsT for G
    wot_d = nc.dram_tensor("wot", [32, C], BF16, kind="ExternalInput")       # (gamma*w_o)^T
    out_d = nc.dram_tensor("out", [BPC, C, HW], F32, kind="ExternalOutput")
    srow_d = nc.dram_tensor("srow", [BPC, HW], F32)   # sumexp bounce (internal)
    rd_d = nc.dram_tensor("rd", [BPC, HW], F32)       # recip bounce (internal)

    with tile.TileContext(nc) as tc, ExitStack() as ctx:
        consts = ctx.enter_context(tc.tile_pool(name="consts", bufs=1))
        wpgt_sb = consts.tile([C, 40], BF16)
        wtheta_sb = consts.tile([8, C], BF16)
        wot_sb = consts.tile([32, C], BF16)
        ident33 = consts.tile([33, 33], BF16)
        nc.sync.dma_start(out=wpgt_sb, in_=wpgt_d[:])
        nc.sync.dma_start(out=wtheta_sb, in_=wtheta_d[:])
        nc.sync.dma_start(out=wot_sb, in_=wot_d[:])
        make_identity(nc, ident33)

        # SBUF pools
        xp = ctx.enter_context(tc.tile_pool(name="xp", bufs=1))
        xbp = ctx.enter_context(tc.tile_pool(name="xbp", bufs=1))
        projp = ctx.enter_context(tc.tile_pool(name="projp", bufs=1))
        t1p = ctx.enter_context(tc.tile_pool(name="t1p", bufs=1))
        gtp = ctx.enter_context(tc.tile_pool(name="gtp", bufs=1))
        hbp = ctx.enter_context(tc.tile_pool(name="hbp", bufs=1))
        Gp = ctx.enter_context(tc.tile_pool(name="Gp", bufs=2))
        hTp = ctx.enter_context(tc.tile_pool(name="hTp", bufs=2))
        expp = ctx.enter_context(tc.tile_pool(name="expp", bufs=6))
        o32p = ctx.enter_context(tc.tile_pool(name="o32p", bufs=2))
        s33p = ctx.enter_context(tc.tile_pool(name="s33p", bufs=2))
        smallp = ctx.enter_context(tc.tile_pool(name="smallp", bufs=4))
        rbp = ctx.enter_context(tc.tile_pool(name="rbp", bufs=2))
        outp = ctx.enter_context(tc.tile_pool(name="outp", bufs=2))

        # PSUM pools (shared across phases; 4 + 2 + 2 = 8 banks)
        psSA = ctx.enter_context(tc.tile_pool(name="psSA", bufs=2, space="PSUM"))
        psGO = ctx.enter_context(tc.tile_pool(name="psGO", bufs=1, space="PSUM"))
        psW = ctx.enter_context(tc.tile_pool(name="psW", bufs=1, space="PSUM"))

        # prefetch both batches
        x_sbs, xb_sbs = [], []
        for b in range(BPC):
            x_sb = xp.tile([C, HW], F32, name=f"x{b}")
            nc.sync.dma_start(out=x_sb, in_=x_d[b])
            xb_sb = xbp.tile([C, HW], BF16, name=f"xb{b}")
            nc.sync.dma_start(out=xb_sb, in_=xb_d[b])
            x_sbs.append(x_sb)
            xb_sbs.append(xb_sb)

        pend = [None]  # (o32, ck, b, recipB) deferred wo emission

        def emit_wo():
            o32, pck, pb, recipB = pend[0]
            pend[0] = None
            wo_ps = psW.tile([C, NCHUNK], F32, name="wo_ps")
            for jj in range(2):
                nc.tensor.matmul(
                    wo_ps[:, jj * 512:(jj + 1) * 512], wot_sb,
                    o32[:, jj * 512:(jj + 1) * 512],
                    start=True, stop=True,
                )
            outc = outp.tile([C, NCHUNK], F32, name="outc")
            nc.vector.tensor_mul(outc, wo_ps, recipB)
            nc.vector.tensor_add(outc, outc, x_sbs[pb][:, pck])
            nc.gpsimd.dma_start(out=out_d[pb, :, pck], in_=outc)

        for b in range(BPC):
            x_sb = x_sbs[b]
            xb_sb = xb_sbs[b]

            # ---- phase A: projections + pipelined pooling, G, hT ----
            proj_sb = projp.tile([40, H, W], BF16)
            t1 = t1p.tile([40, H // 2, W], BF16)
            g_t = gtp.tile([8, H // 2, W // 2], BF16)
            hpb = hbp.tile([33, M], BF16)
            nc.vector.memset(hpb[32:33, :], 1.0)
            hpb3 = hpb[0:32, :].rearrange("c (h w) -> c h w", h=H // 2)
            for k in range(NCH):
                pp = psSA.tile([40, NCHUNK], F32, name="pp", tag="sa")
                for j in range(2):
                    sl = slice(k * NCHUNK + j * 512, k * NCHUNK + (j + 1) * 512)
                    nc.tensor.matmul(
                        pp[:, j * 512:(j + 1) * 512], wpgt_sb, xb_sb[:, sl],
                        start=True, stop=True,
                    )
                hr = slice(16 * k, 16 * (k + 1))   # h rows of this chunk
                pr = slice(8 * k, 8 * (k + 1))     # pooled h rows
                nc.vector.tensor_copy(
                    proj_sb[:, hr, :],
                    pp.rearrange("c (h w) -> c h w", h=16),
                )
                nc.vector.tensor_max(
                    t1[:, pr, :], proj_sb[:, hr, :][:, 0::2, :],
                    proj_sb[:, hr, :][:, 1::2, :],
                )
                nc.vector.tensor_max(
                    g_t[:, pr, :], t1[32:40, pr, 0::2], t1[32:40, pr, 1::2],
                )
                nc.vector.tensor_max(
                    hpb3[:, pr, :], t1[0:32, pr, 0::2], t1[0:32, pr, 1::2],
                )

            # G = w_theta^T @ g  -> [64, M]
            Gps = psGO.tile([C, M], F32, name="Gps", tag="go")
            g_flat = g_t.rearrange("c h w -> c (h w)")
            nc.tensor.matmul(Gps[:, 0:512], wtheta_sb, g_flat[:, 0:512],
                             start=True, stop=True)
            if pend[0] is not None:
                emit_wo()
            nc.tensor.matmul(Gps[:, 512:1024], wtheta_sb, g_flat[:, 512:1024],
                             start=True, stop=True)
            G_sb = Gp.tile([C, M], BF16)
            nc.vector.tensor_copy(G_sb, Gps)

            # hT: transpose h' [33, M] -> [128, MT/2, 2, 34] (DoubleRow layout)
            ht_ps = psGO.tile([128, MT // 2, 2, 34], BF16, name="ht_ps", tag="go")
            for mt in range(MT):
                mt2, j = divmod(mt, 2)
                nc.tensor.transpose(
                    ht_ps[:, mt2, j, 0:33],
                    hpb[:, mt * 128:(mt + 1) * 128],
                    ident33,
                )
            hT8_sb = hTp.tile([128, MT // 2, 2, 48], FP8)
            nc.vector.tensor_copy(hT8_sb[:, :, :, 0:34], ht_ps)

            # ---- phase B: attention per chunk ----
            # PE order per chunk: s0 s1 s2 D0 s3 s4 D1 s5 s6 D2 s7 [wo(prev)] D3
            seq = [("s", 0), ("s", 1), ("s", 2), ("D", 0), ("s", 3),
                   ("s", 4), ("D", 1), ("s", 5), ("s", 6), ("D", 2),
                   ("s", 7), ("wo", None), ("D", 3)]
            for k in range(NCH):
                ck = slice(k * NCHUNK, (k + 1) * NCHUNK)
                o_ps = psGO.tile([33, NCHUNK], F32, name="o_ps", tag="go")
                expTs = {}
                for op, idx in seq:
                    if op == "s":
                        mt = idx
                        mt2, j = divmod(mt, 2)
                        if j == 0:
                            expTs[mt2] = expp.tile(
                                [128, 2, NCHUNK], FP8, name=f"expT{mt2}",
                                tag="exp",
                            )
                        sT = psSA.tile([128, NCHUNK], F32, name="sT", tag="sa")
                        for jj in range(2):
                            sl = slice(
                                k * NCHUNK + jj * 512,
                                k * NCHUNK + (jj + 1) * 512,
                            )
                            nc.tensor.matmul(
                                sT[:, jj * 512:(jj + 1) * 512],
                                G_sb[:, mt * 128:(mt + 1) * 128],
                                xb_sb[:, sl], start=True, stop=True,
                            )
                        nc.scalar.activation(
                            expTs[mt2][:, j, :], sT,
                            func=mybir.ActivationFunctionType.Exp,
                        )
                    elif op == "D":
                        mt2 = idx
                        for jj in range(2):
                            nc.tensor.matmul(
                                o_ps[:, jj * 512:(jj + 1) * 512],
                                hT8_sb[:, mt2, :, 0:33],
                                expTs[mt2][:, :, jj * 512:(jj + 1) * 512],
                                start=(mt2 == 0), stop=(mt2 == MT // 2 - 1),
                                perf_mode=mybir.MatmulPerfMode.DoubleRow,
                            )
                    elif pend[0] is not None:
                        emit_wo()
                # post-chunk: o copy for wo + reciprocal chain (off PE path)
                o32 = o32p.tile([32, NCHUNK], BF16, name="o32")
                nc.vector.tensor_copy(o32, o_ps[0:32, :])
                s33 = s33p.tile([33, NCHUNK], F32, name="s33")
                nc.vector.tensor_copy(s33[32:33, :], o_ps[32:33, :])
                nc.sync.dma_start(out=srow_d[b, ck], in_=s33[32:33, :])
                rs = smallp.tile([128, NCHUNK // 128], F32, name="rs")
                nc.sync.dma_start(
                    out=rs,
                    in_=srow_d[b, ck].rearrange("(p i) -> p i", p=128),
                )
                rr = smallp.tile([128, NCHUNK // 128], F32, name="rr")
                nc.vector.reciprocal(rr, rs)
                nc.sync.dma_start(
                    out=rd_d[b, ck].rearrange("(p i) -> p i", p=128),
                    in_=rr,
                )
                recipB = rbp.tile([C, NCHUNK], F32, name="recipB")
                rd_ck = rd_d[b, ck]
                nc.sync.dma_start(
                    out=recipB,
                    in_=bass.AP(
                        tensor=rd_ck.tensor, offset=rd_ck.offset,
                        ap=[[0, C]] + list(rd_ck.ap),
                    ),
                )
                pend[0] = (o32, ck, b, recipB)
        emit_wo()

    if not nc.is_finalized():
        nc.finalize()
    return nc


_NC_CACHE = {}


def _run(inputs: dict, trace: bool = False):
    if "nc" not in _NC_CACHE:
        _NC_CACHE["nc"] = _build_nc()
    nc = _NC_CACHE["nc"]

    import ml_dtypes

    x = np.ascontiguousarray(inputs["x"], dtype=np.float32).reshape(B, C, HW)
    xb16 = x.astype(ml_dtypes.bfloat16)
    wpgt = np.ascontiguousarray(
        np.concatenate([inputs["w_g"], inputs["w_phi"]], axis=0).T.astype(
            ml_dtypes.bfloat16
        )
    )
    wtheta = np.ascontiguousarray(
        np.asarray(inputs["w_theta"]).astype(ml_dtypes.bfloat16)
    )
    wot = np.ascontiguousarray(
        (float(inputs["gamma"][0]) * inputs["w_o"]).T.astype(ml_dtypes.bfloat16)
    )

    in_maps = []
    for i in range(NCORES):
        in_maps.append({
            "x": np.ascontiguousarray(x[i * BPC:(i + 1) * BPC]),
            "xb": np.ascontiguousarray(xb16[i * BPC:(i + 1) * BPC]),
            "wpgt": wpgt,
            "wtheta": wtheta,
            "wot": wot,
        })

    res = run_bass_kernel_spmd(nc, in_maps, list(range(NCORES)), trace=trace)
    out = np.concatenate([r["out"] for r in res.results], axis=0)
    return out.reshape(B, C, H, W).astype(np.float32), res


def kernel(**inputs):
    out, _ = _run(inputs, trace=False)
    return out
